# revision 7
# baseline (speedup 1.0000x reference)
"""Trainium2 Bass kernel for nn_HarmonicOscillatorOrbitals.

out[b, i, k] = exp(-s^2/2) * H_k(s), s = omega * x[b, i, 0], k = 0..31.

Data-parallel over 8 cores on the batch axis; per core [128 part, W=2048]
scalars, 32 Hermite orders each.  The three-term recurrence runs as a
normalized fp16 chain psi_k = G_k / D_k with D_k = (2/abar_k) D_{k-1} and
abar_k a power of two, so t_bar = abar*s16 is an exact scaling of
s16 = fp16(s) and all BTIL/D scalars are exact in f32.  Per order k:
    q_k = t_bar * psi_{k-1}                  DVE tensor_tensor (fp16 2x mode)
    psi_k = q_k - r_k                        DVE tensor_sub (2x mode), with
    r_k = BTIL_k*psi_{k-2} prepped one order ahead by ACT Copy-w/-scale (fp16)
  cast out_k = D_k*psi_k -> bf16: cols [0,ZC) DVE tensor_scalar (4x mode),
    cols [ZC,W) ACT Copy-with-scale
GPSIMD is deliberately unused for compute: it shares an exclusive SBUF port
pair with DVE's second read port, so overlapping it with DVE tensor_tensor
traffic stretches both engines (~1.75x measured).  DVE and ACT run
co-saturated and balanced at ~3.0 us/order.
Startup: x DMA'd in two halves with DVE prep (s16/twoS/tB/x^2) pipelined per
half; env = Exp(x^2 * (-om^2/2)) avoids the ACT Square table set entirely
(one table load); omega variants [om, om/2, 2om, -om^2/2] are host-prepared
per partition.  Order-0 output (D_0 = 1) is a pure fp16->bf16 SWDGE DMA-cast;
order 1 is seeded as psi_1 = (2 s16) * env so D_1 = 1 too.
The fp16 rounding of s16 is a smooth phase error; it is corrected for the
only order where it matters at the 2e-2 gate (k=31, the global-max order) via
d(G_31)/ds = 62 G_30 - s G_31 using delta = s - s16, pipelined with the final
cast + DMA in column halves.  Output is bf16 (16 MB/core, halving HBM write
traffic); the host upcasts to f32 during unshard.  Measured global rel err
1.186e-2 (deterministic) vs the 2e-2 gate; HW exec ~126 us at nominal clock
(~148 us when the chip sits in its 1.2x-slower DVFS state); baseline f32
kernel: 250 us.
"""

from contextlib import ExitStack

import numpy as np

import concourse.bacc as bacc
import concourse.mybir as mybir
import concourse.tile as tile
from concourse.bass_utils import run_bass_kernel_spmd

F32 = mybir.dt.float32
F16 = mybir.dt.float16
BF16 = mybir.dt.bfloat16
AF = mybir.ActivationFunctionType
ALU = mybir.AluOpType

NJ = 32
N_CORES = 8
B = 65536
BC = B // N_CORES
W = BC * NJ // 128           # 2048
H = W // 2

ZC = 1280                    # DVE-TS cast stripe cols [0, ZC); ACT casts [ZC, W)
OCH = 8

# Normalization: abar[1]=2 (D_1=1, psi_1 = 2 s env), abar=1 for k in 2..4,
# abar=1/2 for k>=5.  All pow2 -> exact.
ABAR = [None, 2.0] + [1.0] * 3 + [0.5] * 27
D = [1.0] * NJ
for _k in range(1, NJ):
    D[_k] = (2.0 / ABAR[_k]) * D[_k - 1]
BTIL = [0.0] * NJ
for _k in range(2, NJ):
    BTIL[_k] = 2.0 * (_k - 1) * D[_k - 2] / D[_k]
C31 = 62.0 * D[30] / D[31]


def _build(zc=ZC):
    nc = bacc.Bacc("TRN2", target_bir_lowering=False, debug=False)
    x_d = nc.dram_tensor("x", [128, W], F32, kind="ExternalInput").ap()
    om_d = nc.dram_tensor("om", [128, 4], F32, kind="ExternalInput").ap()
    out_d = nc.dram_tensor("out", [128, NJ * W], BF16, kind="ExternalOutput").ap()

    with tile.TileContext(nc) as tc, ExitStack() as ctx:
        cpool = ctx.enter_context(tc.tile_pool(name="const", bufs=1))
        ppool = ctx.enter_context(tc.tile_pool(name="psi", bufs=4))
        qpool = ctx.enter_context(tc.tile_pool(name="q", bufs=2))
        rpool = ctx.enter_context(tc.tile_pool(name="r", bufs=3))
        opool = ctx.enter_context(tc.tile_pool(name="ob", bufs=2))

        # x first (longest pole), then host-prepared per-partition omega
        # variants [om, om/2, 2om, -om^2/2]
        xr = cpool.tile([128, W], F32, name="xr")
        nc.sync.dma_start(xr[:, 0:H], x_d[:, 0:H])
        nc.sync.dma_start(xr[:, H:W], x_d[:, H:W])
        om_t = cpool.tile([128, 4], F32, name="om_t")
        nc.sync.dma_start(om_t[:, :], om_d[:, :])
        omv = om_t[:, 0:1]

        obufs = {}

        def oslice(k):
            blk = k // OCH
            if blk not in obufs:
                obufs[blk] = opool.tile(
                    [128, OCH * W], BF16, name=f"ob{blk}", tag="ob"
                )
            return obufs[blk][:, (k % OCH) * W : (k % OCH + 1) * W]

        def flush(a, b):
            blk = a // OCH
            nc.sync.dma_start(
                out_d[:, a * W : b * W],
                obufs[blk][:, (a % OCH) * W : ((b - 1) % OCH + 1) * W],
            )

        # DVE prep in halves as x lands; ACT only does env (one table set)
        s16 = cpool.tile([128, W], F16, name="s16")
        twoS = cpool.tile([128, W], F16, name="twoS")
        tB = cpool.tile([128, W], F16, name="tB")
        xx = cpool.tile([128, W], F32, name="xx")
        env = cpool.tile([128, W], F16, name="env")  # = psi_0
        for a, b in ((0, H), (H, W)):
            nc.vector.tensor_scalar(s16[:, a:b], xr[:, a:b], omv, None, ALU.mult)
            nc.vector.tensor_mul(xx[:, a:b], xr[:, a:b], xr[:, a:b])
            nc.scalar.activation(env[:, a:b], xx[:, a:b], AF.Exp, scale=om_t[:, 3:4])
            nc.vector.tensor_scalar(twoS[:, a:b], xr[:, a:b], om_t[:, 2:3], None, ALU.mult)
            nc.vector.tensor_scalar(tB[:, a:b], xr[:, a:b], om_t[:, 1:2], None, ALU.mult)

        # corr inputs (DVE prep slack): d = s - s16, ds = d*s16
        d16 = cpool.tile([128, W], F16, name="d16")
        nc.vector.scalar_tensor_tensor(
            d16[:, :], xr[:, :], omv, s16[:, :], ALU.mult, ALU.subtract
        )
        ds16 = cpool.tile([128, W], F16, name="ds16")
        nc.vector.tensor_mul(ds16[:, :], d16[:, :], s16[:, :])

        # order 0: pure convert fp16 -> bf16 via SWDGE DMA-cast (D_0 = 1)
        nc.gpsimd.dma_start(out_d[:, 0:W], env[:, :])

        psi_m1 = cpool.tile([128, W], F16, name="psi1")  # psi_1 = 2 s env (D_1=1)
        nc.vector.tensor_mul(psi_m1[:, :], twoS[:, :], env[:, :])
        nc.gpsimd.dma_start(out_d[:, W : 2 * W], psi_m1[:, :])
        psi_m2 = env

        psi30 = psi31 = None
        r_tiles = {}
        r_tiles[2] = rpool.tile([128, W], F16, name="r2", tag="r")
        nc.scalar.activation(
            r_tiles[2][:, :], psi_m2[:, :], AF.Copy, scale=float(BTIL[2])
        )
        for k in range(2, NJ):
            t = s16 if k <= 4 else tB
            q = qpool.tile([128, W], F16, name=f"q{k}", tag="q")
            nc.vector.tensor_mul(q[:, :], t[:, :], psi_m1[:, :])
            psi_k = ppool.tile([128, W], F16, name=f"psi{k}", tag="psi")
            if k + 1 < NJ:
                r_tiles[k + 1] = rpool.tile([128, W], F16, name=f"r{k+1}", tag="r")
                nc.scalar.activation(
                    r_tiles[k + 1][:, :],
                    psi_m1[:, :],
                    AF.Copy,
                    scale=float(BTIL[k + 1]),
                )
            nc.vector.tensor_sub(psi_k[:, :], q[:, :], r_tiles[k][:, :])
            del r_tiles[k]
            if k < 31:
                # cast split: DVE TS 4x on cols [0, zc), ACT on [zc, W)
                nc.vector.tensor_scalar(
                    oslice(k)[:, 0:zc], psi_k[:, 0:zc], float(D[k]), None, ALU.mult
                )
                nc.scalar.activation(
                    oslice(k)[:, zc:], psi_k[:, zc:], AF.Copy, scale=float(D[k])
                )
            if k == 30:
                psi30 = psi_k
            if k == 31:
                psi31 = psi_k
            psi_m2, psi_m1 = psi_m1, psi_k
            for (kf, a, b) in (
                (7, 2, 8), (15, 8, 16), (21, 16, 22), (23, 22, 24),
                (26, 24, 27), (29, 27, 30), (30, 30, 31),
            ):
                if k == kf:
                    flush(a, b)

        # order-31 phase correction, pipelined in column halves:
        #   c31 = (psi31 + C31*(d*psi30)) - (d*s)*psi31
        cA = cpool.tile([128, W], F16, name="corrA")
        cB = cpool.tile([128, W], F16, name="corrB")
        cC = cpool.tile([128, W], F16, name="corrC")
        c31 = cpool.tile([128, W], F16, name="corr31")
        for a, b in ((0, H), (H, W)):
            nc.vector.tensor_mul(cA[:, a:b], d16[:, a:b], psi30[:, a:b])
            nc.vector.tensor_mul(cB[:, a:b], ds16[:, a:b], psi31[:, a:b])
            nc.vector.scalar_tensor_tensor(
                cC[:, a:b], cA[:, a:b], float(C31), psi31[:, a:b],
                ALU.mult, ALU.add,
            )
            nc.vector.tensor_sub(c31[:, a:b], cC[:, a:b], cB[:, a:b])
            if a == 0:
                nc.vector.tensor_scalar(
                    oslice(31)[:, a:b], c31[:, a:b], float(D[31]), None, ALU.mult
                )
            else:
                nc.scalar.activation(
                    oslice(31)[:, a:b], c31[:, a:b], AF.Copy, scale=float(D[31])
                )
            blk = 31 // OCH
            nc.sync.dma_start(
                out_d[:, 31 * W + a : 31 * W + b],
                obufs[blk][:, (31 % OCH) * W + a : (31 % OCH) * W + b],
            )

    nc.compile()
    return nc


_CACHED_NC = None


def kernel(x: np.ndarray, omega_kernel: np.ndarray, **run_kwargs) -> np.ndarray:
    global _CACHED_NC
    assert x.shape == (B, NJ, 1) and omega_kernel.shape == (1, 1)
    x = np.ascontiguousarray(x, np.float32)
    om = np.ascontiguousarray(omega_kernel, np.float32)

    if _CACHED_NC is None:
        _CACHED_NC = _build()
    nc = _CACHED_NC

    o = float(om[0, 0])
    om_vec = np.tile(
        np.array([[o, 0.5 * o, 2.0 * o, -0.5 * o * o]], np.float32), (128, 1)
    )
    in_maps = [
        {"x": x[c * BC : (c + 1) * BC].reshape(128, W), "om": om_vec}
        for c in range(N_CORES)
    ]
    res = run_bass_kernel_spmd(nc, in_maps, core_ids=list(range(N_CORES)), **run_kwargs)
    full = np.empty((B, NJ, NJ), np.float32)
    for c in range(N_CORES):
        arr = np.asarray(res.results[c]["out"]).astype(np.float32)
        arr = arr.reshape(128, NJ, W).transpose(0, 2, 1).reshape(BC, NJ, NJ)
        full[c * BC : (c + 1) * BC] = arr
    if run_kwargs:
        return full, res
    return full


# revision 8
# speedup vs baseline: 1.1837x; 1.1837x over previous
"""Trainium2 Bass kernel for nn_HarmonicOscillatorOrbitals.

out[b, i, k] = exp(-s^2/2) * H_k(s), s = omega * x[b, i, 0], k = 0..31.

Data-parallel over 8 cores on the batch axis; per core [128 part, W=2048]
scalars, 32 Hermite orders each.  The three-term recurrence runs as a
normalized fp16 chain psi_k = G_k / D_k with D_k = (2/abar_k) D_{k-1} and
abar_k a power of two, so t_bar = abar*s16 is an exact scaling of
s16 = fp16(s) and all BTIL/D scalars are exact in f32.  Per order k:
    q_k = t_bar * psi_{k-1}                  DVE tensor_tensor (fp16 2x mode)
    psi_k = q_k - r_k                        DVE tensor_sub (2x mode), with
    r_k = BTIL_k*psi_{k-2} prepped one order ahead by ACT Copy-w/-scale (fp16)
  cast out_k = D_k*psi_k -> bf16: cols [0,ZC) DVE tensor_scalar (4x mode),
    cols [ZC,W) ACT Copy-with-scale
GPSIMD is deliberately unused for compute: it shares an exclusive SBUF port
pair with DVE's second read port, so overlapping it with DVE tensor_tensor
traffic stretches both engines (~1.75x measured).  DVE and ACT run
co-saturated and balanced at ~3.0 us/order.
Startup: x DMA'd in two halves with DVE prep (s16/twoS/tB/x^2) pipelined per
half; env = Exp(x^2 * (-om^2/2)) avoids the ACT Square table set entirely
(one table load); omega variants [om, om/2, 2om, -om^2/2] are host-prepared
per partition.  Order-0 output (D_0 = 1) is a pure fp16->bf16 SWDGE DMA-cast;
order 1 is seeded as psi_1 = (2 s16) * env so D_1 = 1 too.
The fp16 rounding of s16 is a smooth phase error; it is corrected for the
only order where it matters at the 2e-2 gate (k=31, the global-max order) via
d(G_31)/ds = 62 G_30 - s G_31 using delta = s - s16, pipelined with the final
cast + DMA in column halves.  Output is bf16 (16 MB/core, halving HBM write
traffic); the host upcasts to f32 during unshard.  Measured global rel err
1.186e-2 (deterministic) vs the 2e-2 gate; HW exec ~126 us at nominal clock
(~148 us when the chip sits in its 1.2x-slower DVFS state); baseline f32
kernel: 250 us.
"""

from contextlib import ExitStack

import numpy as np

import concourse.bacc as bacc
import concourse.mybir as mybir
import concourse.tile as tile
from concourse.bass_utils import run_bass_kernel_spmd

F32 = mybir.dt.float32
F16 = mybir.dt.float16
BF16 = mybir.dt.bfloat16
AF = mybir.ActivationFunctionType
ALU = mybir.AluOpType

NJ = 32
N_CORES = 8
B = 65536
BC = B // N_CORES
W = BC * NJ // 128           # 2048
H = W // 2

ZC = 1120                    # DVE-TS cast stripe cols [0, ZC); ACT casts [ZC, W)
OCH = 8

# Normalization: abar[1]=2 (D_1=1, psi_1 = 2 s env), abar=1 for k in 2..4,
# abar=1/2 for k>=5.  All pow2 -> exact.
ABAR = [None, 2.0] + [1.0] * 3 + [0.5] * 27
D = [1.0] * NJ
for _k in range(1, NJ):
    D[_k] = (2.0 / ABAR[_k]) * D[_k - 1]
BTIL = [0.0] * NJ
for _k in range(2, NJ):
    BTIL[_k] = 2.0 * (_k - 1) * D[_k - 2] / D[_k]
C31 = 62.0 * D[30] / D[31]


def _build(zc=ZC):
    nc = bacc.Bacc("TRN2", target_bir_lowering=False, debug=False)
    x_d = nc.dram_tensor("x", [128, W], F32, kind="ExternalInput").ap()
    om_d = nc.dram_tensor("om", [128, 4], F32, kind="ExternalInput").ap()
    out_d = nc.dram_tensor("out", [128, NJ * W], BF16, kind="ExternalOutput").ap()

    with tile.TileContext(nc) as tc, ExitStack() as ctx:
        cpool = ctx.enter_context(tc.tile_pool(name="const", bufs=1))
        ppool = ctx.enter_context(tc.tile_pool(name="psi", bufs=4))
        qpool = ctx.enter_context(tc.tile_pool(name="q", bufs=2))
        rpool = ctx.enter_context(tc.tile_pool(name="r", bufs=3))
        opool = ctx.enter_context(tc.tile_pool(name="ob", bufs=2))

        # x first (longest pole), then host-prepared per-partition omega
        # variants [om, om/2, 2om, -om^2/2]
        xr = cpool.tile([128, W], F32, name="xr")
        nc.sync.dma_start(xr[:, 0:H], x_d[:, 0:H])
        nc.sync.dma_start(xr[:, H:W], x_d[:, H:W])
        om_t = cpool.tile([128, 4], F32, name="om_t")
        nc.sync.dma_start(om_t[:, :], om_d[:, :])
        omv = om_t[:, 0:1]

        obufs = {}

        def oslice(k):
            blk = k // OCH
            if blk not in obufs:
                obufs[blk] = opool.tile(
                    [128, OCH * W], BF16, name=f"ob{blk}", tag="ob"
                )
            return obufs[blk][:, (k % OCH) * W : (k % OCH + 1) * W]

        def flush(a, b):
            blk = a // OCH
            nc.sync.dma_start(
                out_d[:, a * W : b * W],
                obufs[blk][:, (a % OCH) * W : ((b - 1) % OCH + 1) * W],
            )

        # prep in halves as x lands: DVE does xx/s16 (xx first: it gates env),
        # ACT does env then the exact pow2 scale-copies twoS = 2*s, tB = s/2
        s16 = cpool.tile([128, W], F16, name="s16")
        twoS = cpool.tile([128, W], F16, name="twoS")
        tB = cpool.tile([128, W], F16, name="tB")
        xx = cpool.tile([128, W], F32, name="xx")
        env = cpool.tile([128, W], F16, name="env")  # = psi_0
        for a, b in ((0, H), (H, W)):
            nc.vector.tensor_mul(xx[:, a:b], xr[:, a:b], xr[:, a:b])
            nc.vector.tensor_scalar(s16[:, a:b], xr[:, a:b], omv, None, ALU.mult)
            nc.scalar.activation(env[:, a:b], xx[:, a:b], AF.Exp, scale=om_t[:, 3:4])
        nc.scalar.activation(twoS[:, :], xr[:, :], AF.Copy, scale=om_t[:, 2:3])
        nc.scalar.activation(tB[:, :], xr[:, :], AF.Copy, scale=om_t[:, 1:2])

        # corr inputs (DVE prep slack): d = s - s16, ds = d*s16
        d16 = cpool.tile([128, W], F16, name="d16")
        nc.vector.scalar_tensor_tensor(
            d16[:, :], xr[:, :], omv, s16[:, :], ALU.mult, ALU.subtract
        )
        ds16 = cpool.tile([128, W], F16, name="ds16")
        nc.vector.tensor_mul(ds16[:, :], d16[:, :], s16[:, :])

        # order 0: pure convert fp16 -> bf16 via SWDGE DMA-cast (D_0 = 1)
        nc.gpsimd.dma_start(out_d[:, 0:W], env[:, :])

        psi_m1 = cpool.tile([128, W], F16, name="psi1")  # psi_1 = 2 s env (D_1=1)
        nc.vector.tensor_mul(psi_m1[:, :], twoS[:, :], env[:, :])
        nc.gpsimd.dma_start(out_d[:, W : 2 * W], psi_m1[:, :])
        psi_m2 = env

        psi30 = psi31 = None
        r_tiles = {}
        r_tiles[2] = rpool.tile([128, W], F16, name="r2", tag="r")
        nc.scalar.activation(
            r_tiles[2][:, :], psi_m2[:, :], AF.Copy, scale=float(BTIL[2])
        )
        for k in range(2, NJ):
            t = s16 if k <= 4 else tB
            q = qpool.tile([128, W], F16, name=f"q{k}", tag="q")
            nc.vector.tensor_mul(q[:, :], t[:, :], psi_m1[:, :])
            psi_k = ppool.tile([128, W], F16, name=f"psi{k}", tag="psi")
            if k + 1 < NJ:
                r_tiles[k + 1] = rpool.tile([128, W], F16, name=f"r{k+1}", tag="r")
                nc.scalar.activation(
                    r_tiles[k + 1][:, :],
                    psi_m1[:, :],
                    AF.Copy,
                    scale=float(BTIL[k + 1]),
                )
            nc.vector.tensor_sub(psi_k[:, :], q[:, :], r_tiles[k][:, :])
            del r_tiles[k]
            if k < 31:
                # cast split: DVE TS 4x on cols [0, zc), ACT on [zc, W)
                nc.vector.tensor_scalar(
                    oslice(k)[:, 0:zc], psi_k[:, 0:zc], float(D[k]), None, ALU.mult
                )
                nc.scalar.activation(
                    oslice(k)[:, zc:], psi_k[:, zc:], AF.Copy, scale=float(D[k])
                )
            if k == 30:
                psi30 = psi_k
            if k == 31:
                psi31 = psi_k
            psi_m2, psi_m1 = psi_m1, psi_k
            for (kf, a, b) in (
                (7, 2, 8), (15, 8, 16), (21, 16, 22), (23, 22, 24),
                (26, 24, 27), (29, 27, 30), (30, 30, 31),
            ):
                if k == kf:
                    flush(a, b)

        # order-31 phase correction, pipelined in column halves:
        #   c31 = (psi31 + C31*(d*psi30)) - (d*s)*psi31
        cA = cpool.tile([128, W], F16, name="corrA")
        cB = cpool.tile([128, W], F16, name="corrB")
        cC = cpool.tile([128, W], F16, name="corrC")
        c31 = cpool.tile([128, W], F16, name="corr31")
        for a, b in ((0, H), (H, W)):
            nc.vector.tensor_mul(cA[:, a:b], d16[:, a:b], psi30[:, a:b])
            nc.vector.tensor_mul(cB[:, a:b], ds16[:, a:b], psi31[:, a:b])
            nc.vector.scalar_tensor_tensor(
                cC[:, a:b], cA[:, a:b], float(C31), psi31[:, a:b],
                ALU.mult, ALU.add,
            )
            nc.vector.tensor_sub(c31[:, a:b], cC[:, a:b], cB[:, a:b])
            if a == 0:
                nc.vector.tensor_scalar(
                    oslice(31)[:, a:b], c31[:, a:b], float(D[31]), None, ALU.mult
                )
            else:
                nc.scalar.activation(
                    oslice(31)[:, a:b], c31[:, a:b], AF.Copy, scale=float(D[31])
                )
            blk = 31 // OCH
            nc.sync.dma_start(
                out_d[:, 31 * W + a : 31 * W + b],
                obufs[blk][:, (31 % OCH) * W + a : (31 % OCH) * W + b],
            )

    nc.compile()
    return nc


_CACHED_NC = None


def kernel(x: np.ndarray, omega_kernel: np.ndarray, **run_kwargs) -> np.ndarray:
    global _CACHED_NC
    assert x.shape == (B, NJ, 1) and omega_kernel.shape == (1, 1)
    x = np.ascontiguousarray(x, np.float32)
    om = np.ascontiguousarray(omega_kernel, np.float32)

    if _CACHED_NC is None:
        _CACHED_NC = _build()
    nc = _CACHED_NC

    o = float(om[0, 0])
    om_vec = np.tile(
        np.array([[o, 0.5 * o, 2.0 * o, -0.5 * o * o]], np.float32), (128, 1)
    )
    in_maps = [
        {"x": x[c * BC : (c + 1) * BC].reshape(128, W), "om": om_vec}
        for c in range(N_CORES)
    ]
    res = run_bass_kernel_spmd(nc, in_maps, core_ids=list(range(N_CORES)), **run_kwargs)
    full = np.empty((B, NJ, NJ), np.float32)
    for c in range(N_CORES):
        arr = np.asarray(res.results[c]["out"]).astype(np.float32)
        arr = arr.reshape(128, NJ, W).transpose(0, 2, 1).reshape(BC, NJ, NJ)
        full[c * BC : (c + 1) * BC] = arr
    if run_kwargs:
        return full, res
    return full


# revision 9
# speedup vs baseline: 1.1857x; 1.0017x over previous
"""Trainium2 Bass kernel for nn_HarmonicOscillatorOrbitals.

out[b, i, k] = exp(-s^2/2) * H_k(s), s = omega * x[b, i, 0], k = 0..31.

Data-parallel over 8 cores on the batch axis; per core [128 part, W=2048]
scalars, 32 Hermite orders each.  The three-term recurrence runs as a
normalized fp16 chain psi_k = G_k / D_k with D_k = (2/abar_k) D_{k-1} and
abar_k a power of two, so t_bar = abar*s16 is an exact scaling of
s16 = fp16(s) and all BTIL/D scalars are exact in f32.  Per order k:
    q_k = t_bar * psi_{k-1}                  DVE tensor_tensor (fp16 2x mode)
    psi_k = q_k - r_k                        DVE tensor_sub (2x mode), with
    r_k = BTIL_k*psi_{k-2} prepped one order ahead by ACT Copy-w/-scale (fp16)
  cast out_k = D_k*psi_k -> bf16: cols [0,ZC) DVE tensor_scalar (4x mode),
    cols [ZC,W) ACT Copy-with-scale
GPSIMD is deliberately unused for compute: it shares an exclusive SBUF port
pair with DVE's second read port, so overlapping it with DVE tensor_tensor
traffic stretches both engines (~1.75x measured).  DVE and ACT run
co-saturated and balanced at ~3.0 us/order.
Startup: x DMA'd in two halves; DVE computes x^2/s16 per half while ACT does
env and the exact pow2 scale-copies twoS = 2*s16, tB = s16/2;
env = Exp(x^2 * (-om^2/2)) avoids the ACT Square table set entirely
(one table load); omega variants [om, om/2, 2om, -om^2/2] are host-prepared
per partition.  Order-0 output (D_0 = 1) is a pure fp16->bf16 SWDGE DMA-cast;
order 1 is seeded as psi_1 = (2 s16) * env so D_1 = 1 too.
The fp16 rounding of s16 is a smooth phase error; it is corrected for the
only order where it matters at the 2e-2 gate (k=31, the global-max order) via
d(G_31)/ds = 62 G_30 - s G_31 using delta = s - s16, pipelined with the final
cast + DMA in column halves.  Output is bf16 (16 MB/core, halving HBM write
traffic); the host upcasts to f32 during unshard.  Measured global rel err
1.186e-2 (deterministic) vs the 2e-2 gate; HW exec ~123 us at nominal clock
(~146 us when the chip sits in its 1.2x-slower DVFS state); baseline f32
kernel: 250 us.
"""

from contextlib import ExitStack

import numpy as np

import concourse.bacc as bacc
import concourse.mybir as mybir
import concourse.tile as tile
from concourse.bass_utils import run_bass_kernel_spmd

F32 = mybir.dt.float32
F16 = mybir.dt.float16
BF16 = mybir.dt.bfloat16
AF = mybir.ActivationFunctionType
ALU = mybir.AluOpType

NJ = 32
N_CORES = 8
B = 65536
BC = B // N_CORES
W = BC * NJ // 128           # 2048
H = W // 2

ZC = 1120                    # DVE-TS cast stripe cols [0, ZC); ACT casts [ZC, W)
OCH = 8

# Normalization: abar[1]=2 (D_1=1, psi_1 = 2 s env), abar=1 for k in 2..4,
# abar=1/2 for k>=5.  All pow2 -> exact.
ABAR = [None, 2.0] + [1.0] * 3 + [0.5] * 27
D = [1.0] * NJ
for _k in range(1, NJ):
    D[_k] = (2.0 / ABAR[_k]) * D[_k - 1]
BTIL = [0.0] * NJ
for _k in range(2, NJ):
    BTIL[_k] = 2.0 * (_k - 1) * D[_k - 2] / D[_k]
C31 = 62.0 * D[30] / D[31]


def _build(zc=ZC):
    nc = bacc.Bacc("TRN2", target_bir_lowering=False, debug=False)
    x_d = nc.dram_tensor("x", [128, W], F32, kind="ExternalInput").ap()
    om_d = nc.dram_tensor("om", [128, 4], F32, kind="ExternalInput").ap()
    out_d = nc.dram_tensor("out", [128, NJ * W], BF16, kind="ExternalOutput").ap()

    with tile.TileContext(nc) as tc, ExitStack() as ctx:
        cpool = ctx.enter_context(tc.tile_pool(name="const", bufs=1))
        ppool = ctx.enter_context(tc.tile_pool(name="psi", bufs=4))
        qpool = ctx.enter_context(tc.tile_pool(name="q", bufs=2))
        rpool = ctx.enter_context(tc.tile_pool(name="r", bufs=3))
        opool = ctx.enter_context(tc.tile_pool(name="ob", bufs=2))

        # x first (longest pole), then host-prepared per-partition omega
        # variants [om, om/2, 2om, -om^2/2]
        xr = cpool.tile([128, W], F32, name="xr")
        nc.sync.dma_start(xr[:, 0:H], x_d[:, 0:H])
        nc.sync.dma_start(xr[:, H:W], x_d[:, H:W])
        om_t = cpool.tile([128, 4], F32, name="om_t")
        nc.sync.dma_start(om_t[:, :], om_d[:, :])
        omv = om_t[:, 0:1]

        obufs = {}

        def oslice(k):
            blk = k // OCH
            if blk not in obufs:
                obufs[blk] = opool.tile(
                    [128, OCH * W], BF16, name=f"ob{blk}", tag="ob"
                )
            return obufs[blk][:, (k % OCH) * W : (k % OCH + 1) * W]

        def flush(a, b):
            blk = a // OCH
            nc.sync.dma_start(
                out_d[:, a * W : b * W],
                obufs[blk][:, (a % OCH) * W : ((b - 1) % OCH + 1) * W],
            )

        # prep in halves as x lands: DVE does xx/s16 (xx first: it gates env),
        # ACT does env then the exact pow2 scale-copies twoS = 2*s, tB = s/2
        s16 = cpool.tile([128, W], F16, name="s16")
        twoS = cpool.tile([128, W], F16, name="twoS")
        tB = cpool.tile([128, W], F16, name="tB")
        xx = cpool.tile([128, W], F32, name="xx")
        env = cpool.tile([128, W], F16, name="env")  # = psi_0
        for a, b in ((0, H), (H, W)):
            nc.vector.tensor_mul(xx[:, a:b], xr[:, a:b], xr[:, a:b])
            nc.vector.tensor_scalar(s16[:, a:b], xr[:, a:b], omv, None, ALU.mult)
            nc.scalar.activation(env[:, a:b], xx[:, a:b], AF.Exp, scale=om_t[:, 3:4])
        nc.scalar.activation(twoS[:, :], xr[:, :], AF.Copy, scale=om_t[:, 2:3])
        nc.scalar.activation(tB[:, :], xr[:, :], AF.Copy, scale=om_t[:, 1:2])

        # corr inputs (DVE prep slack): d = s - s16, ds = d*s16
        d16 = cpool.tile([128, W], F16, name="d16")
        nc.vector.scalar_tensor_tensor(
            d16[:, :], xr[:, :], omv, s16[:, :], ALU.mult, ALU.subtract
        )
        ds16 = cpool.tile([128, W], F16, name="ds16")
        nc.vector.tensor_mul(ds16[:, :], d16[:, :], s16[:, :])

        # order 0: pure convert fp16 -> bf16 via SWDGE DMA-cast (D_0 = 1)
        nc.gpsimd.dma_start(out_d[:, 0:W], env[:, :])

        psi_m1 = cpool.tile([128, W], F16, name="psi1")  # psi_1 = 2 s env (D_1=1)
        nc.vector.tensor_mul(psi_m1[:, :], twoS[:, :], env[:, :])
        nc.gpsimd.dma_start(out_d[:, W : 2 * W], psi_m1[:, :])
        psi_m2 = env

        psi30 = psi31 = None
        r_tiles = {}
        r_tiles[2] = rpool.tile([128, W], F16, name="r2", tag="r")
        nc.scalar.activation(
            r_tiles[2][:, :], psi_m2[:, :], AF.Copy, scale=float(BTIL[2])
        )
        for k in range(2, NJ):
            t = s16 if k <= 4 else tB
            q = qpool.tile([128, W], F16, name=f"q{k}", tag="q")
            nc.vector.tensor_mul(q[:, :], t[:, :], psi_m1[:, :])
            psi_k = ppool.tile([128, W], F16, name=f"psi{k}", tag="psi")
            if k + 1 < NJ:
                r_tiles[k + 1] = rpool.tile([128, W], F16, name=f"r{k+1}", tag="r")
                nc.scalar.activation(
                    r_tiles[k + 1][:, :],
                    psi_m1[:, :],
                    AF.Copy,
                    scale=float(BTIL[k + 1]),
                )
            nc.vector.tensor_sub(psi_k[:, :], q[:, :], r_tiles[k][:, :])
            del r_tiles[k]
            if k < 31:
                # cast split: DVE TS 4x on cols [0, zc), ACT on [zc, W)
                nc.vector.tensor_scalar(
                    oslice(k)[:, 0:zc], psi_k[:, 0:zc], float(D[k]), None, ALU.mult
                )
                nc.scalar.activation(
                    oslice(k)[:, zc:], psi_k[:, zc:], AF.Copy, scale=float(D[k])
                )
            if k == 30:
                psi30 = psi_k
            if k == 31:
                psi31 = psi_k
            psi_m2, psi_m1 = psi_m1, psi_k
            for (kf, a, b) in (
                (7, 2, 8), (15, 8, 16), (21, 16, 22), (23, 22, 24),
                (26, 24, 27), (29, 27, 30), (30, 30, 31),
            ):
                if k == kf:
                    flush(a, b)

        # order-31 phase correction, pipelined in column halves:
        #   c31 = (psi31 + C31*(d*psi30)) - (d*s)*psi31
        cA = cpool.tile([128, W], F16, name="corrA")
        cB = cpool.tile([128, W], F16, name="corrB")
        cC = cpool.tile([128, W], F16, name="corrC")
        c31 = cpool.tile([128, W], F16, name="corr31")
        for a, b in ((0, H), (H, W)):
            nc.vector.tensor_mul(cA[:, a:b], d16[:, a:b], psi30[:, a:b])
            nc.vector.tensor_mul(cB[:, a:b], ds16[:, a:b], psi31[:, a:b])
            nc.vector.scalar_tensor_tensor(
                cC[:, a:b], cA[:, a:b], float(C31), psi31[:, a:b],
                ALU.mult, ALU.add,
            )
            nc.vector.tensor_sub(c31[:, a:b], cC[:, a:b], cB[:, a:b])
            if a == 0:
                nc.vector.tensor_scalar(
                    oslice(31)[:, a:b], c31[:, a:b], float(D[31]), None, ALU.mult
                )
            else:
                nc.scalar.activation(
                    oslice(31)[:, a:b], c31[:, a:b], AF.Copy, scale=float(D[31])
                )
            blk = 31 // OCH
            nc.sync.dma_start(
                out_d[:, 31 * W + a : 31 * W + b],
                obufs[blk][:, (31 % OCH) * W + a : (31 % OCH) * W + b],
            )

    nc.compile()
    return nc


_CACHED_NC = None


def kernel(x: np.ndarray, omega_kernel: np.ndarray, **run_kwargs) -> np.ndarray:
    global _CACHED_NC
    assert x.shape == (B, NJ, 1) and omega_kernel.shape == (1, 1)
    x = np.ascontiguousarray(x, np.float32)
    om = np.ascontiguousarray(omega_kernel, np.float32)

    if _CACHED_NC is None:
        _CACHED_NC = _build()
    nc = _CACHED_NC

    o = float(om[0, 0])
    om_vec = np.tile(
        np.array([[o, 0.5 * o, 2.0 * o, -0.5 * o * o]], np.float32), (128, 1)
    )
    in_maps = [
        {"x": x[c * BC : (c + 1) * BC].reshape(128, W), "om": om_vec}
        for c in range(N_CORES)
    ]
    res = run_bass_kernel_spmd(nc, in_maps, core_ids=list(range(N_CORES)), **run_kwargs)
    full = np.empty((B, NJ, NJ), np.float32)
    for c in range(N_CORES):
        arr = np.asarray(res.results[c]["out"]).astype(np.float32)
        arr = arr.reshape(128, NJ, W).transpose(0, 2, 1).reshape(BC, NJ, NJ)
        full[c * BC : (c + 1) * BC] = arr
    if run_kwargs:
        return full, res
    return full


# revision 10
# speedup vs baseline: 1.2236x; 1.0319x over previous
"""Trainium2 Bass kernel for nn_HarmonicOscillatorOrbitals.

out[b, i, k] = exp(-s^2/2) * H_k(s), s = omega * x[b, i, 0], k = 0..31.

Data-parallel over 8 cores on the batch axis; per core [128 part, W=2048]
scalars, 32 Hermite orders each.  The three-term recurrence runs as a
normalized fp16 chain psi_k = G_k / D_k with D_k = (2/abar_k) D_{k-1} and
abar_k a power of two, so t_bar = abar*s16 is an exact scaling of
s16 = fp16(s) and all BTIL/D scalars are exact in f32.  Per order k:
    q_k = t_bar * psi_{k-1}                  DVE tensor_tensor (fp16 2x mode)
    psi_k = q_k - r_k                        DVE tensor_sub (2x mode), with
    r_k = BTIL_k*psi_{k-2} prepped one order ahead by ACT Copy-w/-scale (fp16)
  cast out_k = D_k*psi_k -> bf16: cols [0,ZC) DVE tensor_scalar (4x mode),
    cols [ZC,W) ACT Copy-with-scale
GPSIMD is deliberately unused for compute: it shares an exclusive SBUF port
pair with DVE's second read port, so overlapping it with DVE tensor_tensor
traffic stretches both engines (~1.75x measured).  DVE and ACT run
co-saturated and balanced at ~3.0 us/order.
Startup: x DMA'd in two halves; DVE computes x^2/s16 per half while ACT does
env and the exact pow2 scale-copies twoS = 2*s16, tB = s16/2;
env = Exp(x^2 * (-om^2/2)) avoids the ACT Square table set entirely
(one table load); omega variants [om, om/2, 2om, -om^2/2] are host-prepared
per partition.  Order-0 output (D_0 = 1) is a pure fp16->bf16 SWDGE DMA-cast;
order 1 is seeded as psi_1 = (2 s16) * env so D_1 = 1 too.
The fp16 rounding of s16 is a smooth phase error; it is corrected for the
only order where it matters at the 2e-2 gate (k=31, the global-max order) via
d(G_31)/ds = 62 G_30 - s G_31 using delta = s - s16, pipelined with the final
cast + DMA in column halves.  Output is bf16 (16 MB/core, halving HBM write
traffic); the host upcasts to f32 during unshard.  Measured global rel err
1.186e-2 (deterministic) vs the 2e-2 gate; HW exec ~123 us at nominal clock
(~146 us when the chip sits in its 1.2x-slower DVFS state); baseline f32
kernel: 250 us.
"""

from contextlib import ExitStack

import numpy as np

import concourse.bacc as bacc
import concourse.mybir as mybir
import concourse.tile as tile
from concourse.bass_utils import run_bass_kernel_spmd

F32 = mybir.dt.float32
F16 = mybir.dt.float16
BF16 = mybir.dt.bfloat16
AF = mybir.ActivationFunctionType
ALU = mybir.AluOpType

NJ = 32
N_CORES = 8
B = 65536
BC = B // N_CORES
W = BC * NJ // 128           # 2048
H = W // 2

ZC = 1120                    # DVE-TS cast stripe cols [0, ZC); ACT casts [ZC, W)
OCH = 8

# Normalization: abar[1]=2 (D_1=1, psi_1 = 2 s env), abar=1 for k in 2..4,
# abar=1/2 for k>=5.  All pow2 -> exact.
ABAR = [None, 2.0] + [1.0] * 3 + [0.5] * 27
D = [1.0] * NJ
for _k in range(1, NJ):
    D[_k] = (2.0 / ABAR[_k]) * D[_k - 1]
BTIL = [0.0] * NJ
for _k in range(2, NJ):
    BTIL[_k] = 2.0 * (_k - 1) * D[_k - 2] / D[_k]
C31 = 62.0 * D[30] / D[31]


def _build(zc=ZC):
    nc = bacc.Bacc("TRN2", target_bir_lowering=False, debug=False)
    x_d = nc.dram_tensor("x", [128, W], F32, kind="ExternalInput").ap()
    om_d = nc.dram_tensor("om", [128, 4], F32, kind="ExternalInput").ap()
    out_d = nc.dram_tensor("out", [128, NJ * W], BF16, kind="ExternalOutput").ap()

    with tile.TileContext(nc) as tc, ExitStack() as ctx:
        cpool = ctx.enter_context(tc.tile_pool(name="const", bufs=1))
        ppool = ctx.enter_context(tc.tile_pool(name="psi", bufs=4))
        qpool = ctx.enter_context(tc.tile_pool(name="q", bufs=2))
        rpool = ctx.enter_context(tc.tile_pool(name="r", bufs=3))
        opool = ctx.enter_context(tc.tile_pool(name="ob", bufs=2))

        # x first (longest pole), then host-prepared per-partition omega
        # variants [om, om/2, 2om, -om^2/2]
        xr = cpool.tile([128, W], F32, name="xr")
        nc.sync.dma_start(xr[:, 0:H], x_d[:, 0:H])
        nc.sync.dma_start(xr[:, H:W], x_d[:, H:W])
        om_t = cpool.tile([128, 4], F32, name="om_t")
        nc.sync.dma_start(om_t[:, :], om_d[:, :])
        omv = om_t[:, 0:1]

        obufs = {}

        def oslice(k):
            blk = k // OCH
            if blk not in obufs:
                obufs[blk] = opool.tile(
                    [128, OCH * W], BF16, name=f"ob{blk}", tag="ob"
                )
            return obufs[blk][:, (k % OCH) * W : (k % OCH + 1) * W]

        def flush(a, b):
            blk = a // OCH
            nc.sync.dma_start(
                out_d[:, a * W : b * W],
                obufs[blk][:, (a % OCH) * W : ((b - 1) % OCH + 1) * W],
            )

        # prep in halves as x lands: DVE does xx/s16 (xx first: it gates env),
        # ACT does env then the exact pow2 scale-copies twoS = 2*s, tB = s/2
        s16 = cpool.tile([128, W], F16, name="s16")
        twoS = cpool.tile([128, W], F16, name="twoS")
        tB = cpool.tile([128, W], F16, name="tB")
        xx = cpool.tile([128, W], F32, name="xx")
        env = cpool.tile([128, W], F16, name="env")  # = psi_0
        for a, b in ((0, H), (H, W)):
            nc.vector.tensor_mul(xx[:, a:b], xr[:, a:b], xr[:, a:b])
            nc.vector.tensor_scalar(s16[:, a:b], xr[:, a:b], omv, None, ALU.mult)
            nc.scalar.activation(env[:, a:b], xx[:, a:b], AF.Exp, scale=om_t[:, 3:4])
        nc.scalar.activation(twoS[:, :], xr[:, :], AF.Copy, scale=om_t[:, 2:3])
        nc.scalar.activation(tB[:, :], xr[:, :], AF.Copy, scale=om_t[:, 1:2])

        # corr inputs (DVE prep slack): d = s - s16, ds = d*s16
        d16 = cpool.tile([128, W], F16, name="d16")
        nc.vector.scalar_tensor_tensor(
            d16[:, :], xr[:, :], omv, s16[:, :], ALU.mult, ALU.subtract
        )
        ds16 = cpool.tile([128, W], F16, name="ds16")
        nc.vector.tensor_mul(ds16[:, :], d16[:, :], s16[:, :])
        # pre-scaled correction inputs (ACT, idle in prep): dC16 = C31*delta and
        # onemds = 1 - delta*s, so the tail correction collapses to
        # c31 = (1 - d*s)*psi31 + dC16*psi30 -- two 2x-mode TT ops per half
        dC16 = cpool.tile([128, W], F16, name="dC16")
        nc.scalar.activation(dC16[:, :], d16[:, :], AF.Copy, scale=float(C31))
        onemds = cpool.tile([128, W], F16, name="onemds")
        nc.scalar.activation(onemds[:, :], ds16[:, :], AF.Copy, scale=-1.0, bias=1.0)

        # order 0: pure convert fp16 -> bf16 via SWDGE DMA-cast (D_0 = 1)
        nc.gpsimd.dma_start(out_d[:, 0:W], env[:, :])

        psi_m1 = cpool.tile([128, W], F16, name="psi1")  # psi_1 = 2 s env (D_1=1)
        nc.vector.tensor_mul(psi_m1[:, :], twoS[:, :], env[:, :])
        nc.gpsimd.dma_start(out_d[:, W : 2 * W], psi_m1[:, :])
        psi_m2 = env

        psi30 = psi31 = None
        r_tiles = {}
        r_tiles[2] = rpool.tile([128, W], F16, name="r2", tag="r")
        nc.scalar.activation(
            r_tiles[2][:, :], psi_m2[:, :], AF.Copy, scale=float(BTIL[2])
        )
        for k in range(2, NJ):
            t = s16 if k <= 4 else tB
            q = qpool.tile([128, W], F16, name=f"q{k}", tag="q")
            nc.vector.tensor_mul(q[:, :], t[:, :], psi_m1[:, :])
            psi_k = ppool.tile([128, W], F16, name=f"psi{k}", tag="psi")
            if k + 1 < NJ:
                r_tiles[k + 1] = rpool.tile([128, W], F16, name=f"r{k+1}", tag="r")
                nc.scalar.activation(
                    r_tiles[k + 1][:, :],
                    psi_m1[:, :],
                    AF.Copy,
                    scale=float(BTIL[k + 1]),
                )
            nc.vector.tensor_sub(psi_k[:, :], q[:, :], r_tiles[k][:, :])
            del r_tiles[k]
            if k < 31:
                # cast split: DVE TS 4x on cols [0, zc), ACT on [zc, W)
                nc.vector.tensor_scalar(
                    oslice(k)[:, 0:zc], psi_k[:, 0:zc], float(D[k]), None, ALU.mult
                )
                nc.scalar.activation(
                    oslice(k)[:, zc:], psi_k[:, zc:], AF.Copy, scale=float(D[k])
                )
            if k == 30:
                psi30 = psi_k
            if k == 31:
                psi31 = psi_k
            psi_m2, psi_m1 = psi_m1, psi_k
            for (kf, a, b) in (
                (7, 2, 8), (15, 8, 16), (21, 16, 22), (23, 22, 24),
                (26, 24, 27), (29, 27, 30), (30, 30, 31),
            ):
                if k == kf:
                    flush(a, b)

        # order-31 phase correction, pipelined in column halves:
        #   c31 = (1 - d*s)*psi31 + (C31*d)*psi30
        cA = cpool.tile([128, W], F16, name="corrA")
        for a, b in ((0, H), (H, W)):
            nc.vector.tensor_mul(cA[:, a:b], dC16[:, a:b], psi30[:, a:b])
        cU = cpool.tile([128, W], F16, name="corrU")
        c31 = cpool.tile([128, W], F16, name="corr31")
        blk = 31 // OCH
        for a, b in ((0, H), (H, W)):
            nc.vector.tensor_mul(cU[:, a:b], onemds[:, a:b], psi31[:, a:b])
            nc.vector.tensor_add(c31[:, a:b], cU[:, a:b], cA[:, a:b])
            if a == 0:
                nc.vector.tensor_scalar(
                    oslice(31)[:, a:b], c31[:, a:b], float(D[31]), None, ALU.mult
                )
            else:
                nc.scalar.activation(
                    oslice(31)[:, a:b], c31[:, a:b], AF.Copy, scale=float(D[31])
                )
            nc.sync.dma_start(
                out_d[:, 31 * W + a : 31 * W + b],
                obufs[blk][:, (31 % OCH) * W + a : (31 % OCH) * W + b],
            )

    nc.compile()
    return nc


_CACHED_NC = None


def kernel(x: np.ndarray, omega_kernel: np.ndarray, **run_kwargs) -> np.ndarray:
    global _CACHED_NC
    assert x.shape == (B, NJ, 1) and omega_kernel.shape == (1, 1)
    x = np.ascontiguousarray(x, np.float32)
    om = np.ascontiguousarray(omega_kernel, np.float32)

    if _CACHED_NC is None:
        _CACHED_NC = _build()
    nc = _CACHED_NC

    o = float(om[0, 0])
    om_vec = np.tile(
        np.array([[o, 0.5 * o, 2.0 * o, -0.5 * o * o]], np.float32), (128, 1)
    )
    in_maps = [
        {"x": x[c * BC : (c + 1) * BC].reshape(128, W), "om": om_vec}
        for c in range(N_CORES)
    ]
    res = run_bass_kernel_spmd(nc, in_maps, core_ids=list(range(N_CORES)), **run_kwargs)
    full = np.empty((B, NJ, NJ), np.float32)
    for c in range(N_CORES):
        arr = np.asarray(res.results[c]["out"]).astype(np.float32)
        arr = arr.reshape(128, NJ, W).transpose(0, 2, 1).reshape(BC, NJ, NJ)
        full[c * BC : (c + 1) * BC] = arr
    if run_kwargs:
        return full, res
    return full
